# revision 21
# baseline (speedup 1.0000x reference)
"""Contrastive loss (SimCLR-style NT-Xent) Trainium2 kernel.

Full inputs z1, z2: [4096, 1024] f32. Output: scalar f32 loss.

Strategy (8 NeuronCores, SPMD, no collectives) — SYMMETRIC-TRIANGLE:
  sim = reps @ reps.T is symmetric, so only the upper triangle of the
  16x16 grid of 512x512 blocks is computed (136 blocks total, 17/core):
  each computed off-diagonal block (bi, bj) serves row-block bi via
  per-row exp sums AND row-block bj via per-column exp sums. This
  nearly halves PE work vs the full row-sharded GEMM.

  - Host: L2-normalize rows of reps = concat(z1, z2) [8192, 1024] f32,
    transpose to repsT [1024, 8192], scale by 256, cast fp8e4m3.
  - Core c owns row-blocks {c, c+8} (512 rows each -> groups A, B). Its
    moving operand is repsT with columns rotated by c*512; in rotated
    coords every core computes the same block positions (SPMD):
      group A (rows c):   column chunks 0..8  (chunk 0 = self-diagonal
                          block; chunk 8 = block (c, c+8), whose local
                          diagonal holds the positive pairs)
      group B (rows c+8): column chunks 8..15 (chunk 8 = self-diagonal)
    Pair coverage: d = bj - bi mod 16 in {0..7} for every bi plus d=8
    for bi < 8 covers each unordered block pair exactly once.
  - Per (block, m-tile of 128 rows): 4 fp8 DoubleRow matmuls (K=1024)
    into one PSUM bank; ACT exp(s*x) to bf16 (bias 0: off-diag values
    land in [e^-2, e^2]). Engine split keeps the PE the only bottleneck:
      ACT: 68 exps only (no accumulator reads).
      DVE: per-row sums (reduce over the 512 columns) of every exp tile;
           for off-diagonal blocks also the 3 adds collapsing the 4 exp
           tiles to one [128, 512] block sum, DMA'd out in bf16 — the
           cross-partition (column) reduction happens on the host in f64.
      PE:  GEMM matmuls only, warmed up with dummy f32 matmuls during
           the initial DMA wait so the p-state ramp happens off-line.
  - Diagonals (self A, positive, self B) extracted raw from PSUM on DVE
    (identity mul + reduce).
  - Schedule 0..8(A), 9..15(B), then the chunk-8 B diagonal block last
    (its b chunk loaded twice): the final block emits no column work, so
    only its own 4 exps + row reduces trail the last matmul.
  - Host: assemble per-row totals T = rowsum - e_self + e_pos in f64,
    loss = mean(ln T - s*pos).
"""

import time
from contextlib import ExitStack

import numpy as np
import ml_dtypes

import concourse.bass as bass
import concourse.tile as tile
from concourse import bacc
from concourse import mybir
from concourse import bass_utils

B = 4096
D = 1024
S = 2 * B            # 8192 rows/cols of sim
NCORES = 8
P = 128
BLK = 512            # block edge (= one PSUM bank of f32)
GRID = S // BLK      # 16
K_TILES = D // P     # 8
N_CHUNKS = GRID      # 16 column chunks of 512
INV_T = 10.0         # 1 / temperature
EPS = 1e-12
FP8_SCALE = 256.0    # input scale: keeps fp8e4m3 operands in their sweet spot
SIM_SCALE = INV_T / (FP8_SCALE * FP8_SCALE)  # exp(SIM_SCALE * raw)
N_WARMUP = 6         # dummy matmuls to ramp the PE p-state during DMA wait

_FP32 = mybir.dt.float32
_FP8 = mybir.dt.float8e4
_BF16 = mybir.dt.bfloat16
_FP8_NP = mybir.dt.np(_FP8)
_BF16_NP = mybir.dt.np(_BF16)


def _build_bass():
    # Bacc (not raw Bass): its compile() runs generate_event_semaphores,
    # which splits multi-semaphore waits into standalone EventSemaphore
    # instructions — engine instructions can encode only one wait.
    nc = bacc.Bacc("TRN2", debug=False, num_devices=NCORES, enable_partition_id=False)
    # Stationary rows (A|B), blocked per kt-pair slab on the host:
    # [slab, p, j, m] so each partition reads 2KB contiguous per slab and
    # the PE can start after slab 0 + the first b piece land.
    a_in = nc.dram_tensor(
        "lhst", [K_TILES // 2, P, 2, 2 * BLK], _FP8, kind="ExternalInput"
    ).ap()
    # Rotated moving operand blocked per 512-col chunk: [ch, p, kt, col],
    # 4KB contiguous per partition per chunk.
    b_in = nc.dram_tensor(
        "brot", [N_CHUNKS, P, K_TILES, BLK], _FP8, kind="ExternalInput"
    ).ap()
    # Raw reductions out; the final combine runs on the host.
    sums_out = nc.dram_tensor("sums", [P, 8, 9], _FP32, kind="ExternalOutput").ap()
    diag_out = nc.dram_tensor("diag", [P, 12], _FP32, kind="ExternalOutput").ap()
    # Off-diagonal block sums (exp tiles summed over the 4 m-tiles);
    # the host reduces over the partition axis.
    bsums_out = nc.dram_tensor(
        "bsums", [15, P, BLK], _BF16, kind="ExternalOutput"
    ).ap()

    # Pre-TileContext const region (same pattern as Bass.__init__'s
    # const_aps): values read by hot-loop instructions with no tracked
    # dependency, so they add no per-instruction sync waits. Hand off with
    # one semaphore to the consumers instead of a full barrier.
    bias_th = nc.alloc_sbuf_tensor("const-f32-zero", [P, 1], _FP32)
    nc.gpsimd.memset(bias_th.ap(), 0.0)
    nc.const_aps.aps[(_FP32, 0.0)] = bias_th.ap()
    ident_th = nc.alloc_sbuf_tensor("identity-f32", [P, P], _FP32)
    nc.gpsimd.memset(ident_th.ap(), 0.0)
    ident_inst = nc.gpsimd.affine_select(
        out=ident_th.ap(),
        in_=ident_th.ap(),
        compare_op=mybir.AluOpType.not_equal,
        fill=1.0,
        base=0,
        pattern=[[-1, P]],
        channel_multiplier=1,
    )
    const_sem = nc.alloc_semaphore("const-ready")
    ident_inst.then_inc(const_sem, 1)
    nc.vector.wait_ge(const_sem, 1)
    nc.scalar.wait_ge(const_sem, 1)
    nc.tensor.wait_ge(const_sem, 1)

    with tile.TileContext(nc) as tc:
        _body(tc, a_in, b_in, sums_out, diag_out, bsums_out, ident_th.ap())
    nc.compile()
    return nc


def _body(tc, a_in, b_in, sums_out, diag_out, bsums_out, ident):
    nc = tc.nc
    AF = mybir.ActivationFunctionType

    ctx = ExitStack()
    singles = ctx.enter_context(tc.tile_pool(name="singles", bufs=1))
    bpool = ctx.enter_context(tc.tile_pool(name="bchunks", bufs=3))
    # 4 GEMM banks: deep PSUM pipeline so matmuls never wait on the
    # ACT/DVE consumers of the bank being recycled.
    pspool = ctx.enter_context(tc.tile_pool(name="psum", bufs=4, space="PSUM"))
    wppool = ctx.enter_context(tc.tile_pool(name="warm", bufs=1, space="PSUM"))
    # Exp tiles: consumed by the DVE row-reduce and (off-diag) the adds.
    epool = ctx.enter_context(tc.tile_pool(name="exps", bufs=8))
    # Pairwise add scratch + per-block sums (live until their DMA).
    apool = ctx.enter_context(tc.tile_pool(name="eadds", bufs=4))
    smpool = ctx.enter_context(tc.tile_pool(name="esums", bufs=3))
    scratch = ctx.enter_context(tc.tile_pool(name="scratch", bufs=8))

    # PE p-state warmup: dummy f32 matmuls on the identity const while the
    # first input DMAs are in flight (output never read). Sized to end
    # right as the first inputs land so the ramp never restarts.
    warm_ps = wppool.tile([P, P], _FP32)
    for _ in range(N_WARMUP):
        nc.tensor.matmul(warm_ps, ident, ident, start=True, stop=True)

    # Resident stationary operand: the core's 1024 rows (A|B) transposed,
    # [p, kt, m]; loaded as 2 half-K slabs so matmuls (kt 0-3) start as
    # soon as the first halves land.
    a_t = singles.tile([P, K_TILES, 2 * BLK], _FP8)
    a_view = a_in.rearrange("s p j m -> p s j m")  # [128, 4, 2, 1024]

    # Per-row partial sums: [p, gmt, slot]; group A (gmt 0-3) slots 0..8
    # = chunks 0..8, group B (gmt 4-7) slots 0..7 = chunks 8..15.
    sums = singles.tile([P, 8, 9], _FP32)
    # Raw (pre-exp, scaled) diagonals: cols 0-3 self A, 4-7 positive,
    # 8-11 self B (by m-tile).
    diag = singles.tile([P, 12], _FP32)

    # Schedule: A-group chunks 0..8, B-group 9..15, then the B diagonal
    # block (chunk 8, loaded a second time) LAST — diag blocks emit no
    # column work, so only their own exp/reduce chain trails the PE.
    schedule = [(ch, 0) for ch in range(9)] + \
               [(ch, 1) for ch in range(9, 16)] + [(8, 1)]
    for step, (ch, g) in enumerate(schedule):
        b_t = bpool.tile([P, K_TILES, BLK], _FP8)
        if step == 0:
            # Descriptor generation is ~0.7us serial per dma_start per
            # engine: spread the four critical first loads (half-K pieces
            # of b chunk 0 and of a) across three engines so they issue
            # concurrently instead of queueing on Sync.
            nc.sync.dma_start(out=b_t[:, 0:4, :], in_=b_in[0][:, 0:4, :])
            nc.scalar.dma_start(out=b_t[:, 4:8, :], in_=b_in[0][:, 4:8, :])
            nc.gpsimd.dma_start(out=a_t[:, 0:4, :], in_=a_view[:, 0:2])
            nc.sync.dma_start(out=a_t[:, 4:8, :], in_=a_view[:, 2:4])
        else:
            nc.sync.dma_start(out=b_t, in_=b_in[ch])
        if step == 9:
            # Group A fully reduced (steps 0-8): ship its sums and diags
            # while group B runs (issue cost hides under compute).
            nc.sync.dma_start(out=diag_out[:, 0:8], in_=diag[:, 0:8])
            nc.sync.dma_start(out=sums_out[:, 0:4, :], in_=sums[:, 0:4, :])
        goff = g * BLK
        is_diag = (ch == 0 and g == 0) or (ch == 8 and g == 1)
        e_ts = []
        for mt in range(4):
            ps = pspool.tile([P, BLK], _FP32)
            for s in range(4):
                nc.tensor.matmul(
                    ps,
                    a_t[:, 2 * s : 2 * s + 2, goff + mt * P : goff + (mt + 1) * P],
                    b_t[:, 2 * s : 2 * s + 2, :],
                    start=(s == 0),
                    stop=(s == 3),
                    perf_mode=mybir.MatmulPerfMode.DoubleRow,
                )
            gmt = g * 4 + mt
            slot = ch if g == 0 else ch - 8
            e_t = epool.tile([P, BLK], _BF16)
            # Row sums: the ACT accumulator (+187ns/tile) and DVE reduce
            # (~465ns/tile) split the 68 tiles so both engines stay well
            # under the PE; the last blocks use ACT so no DVE backlog
            # trails the final matmul.
            if step in (0, 13, 14, 15, 16):
                nc.scalar.activation(
                    out=e_t, in_=ps, func=AF.Exp, bias=0.0, scale=SIM_SCALE,
                    accum_out=sums[:, gmt, slot : slot + 1])
            else:
                nc.scalar.activation(
                    out=e_t, in_=ps, func=AF.Exp, bias=0.0, scale=SIM_SCALE)
                nc.vector.reduce_sum(
                    sums[:, gmt, slot : slot + 1], e_t,
                    axis=mybir.AxisListType.X)
            e_ts.append(e_t)
            # Raw diagonal extraction on DVE straight from PSUM (fused
            # identity-mask multiply + row reduce): the self diagonals
            # (diag blocks) and the positive diagonal (chunk 8, group A).
            dcol = None
            if is_diag:
                dcol = (0 if ch == 0 else 8) + mt
            elif ch == 8 and g == 0:
                dcol = 4 + mt
            if dcol is not None:
                off = mt * P
                diag_t = scratch.tile([P, P], _FP32)
                nc.vector.tensor_mul(diag_t, ps[:, off : off + P], ident)
                nc.vector.reduce_sum(
                    diag[:, dcol : dcol + 1], diag_t, axis=mybir.AxisListType.X
                )
        if not is_diag:
            # Column contribution of this block (rows of block ch+c): sum
            # the 4 exp tiles on DVE, ship [128, 512] bf16; the host does
            # the final partition reduce in f64.
            e01 = apool.tile([P, BLK], _BF16)
            e23 = apool.tile([P, BLK], _BF16)
            esum = smpool.tile([P, BLK], _BF16)
            nc.vector.tensor_add(e01, e_ts[0], e_ts[1])
            nc.vector.tensor_add(e23, e_ts[2], e_ts[3])
            nc.vector.tensor_add(esum, e01, e23)
            nc.sync.dma_start(out=bsums_out[ch - 1], in_=esum)

    # Tail: the remaining outputs, issued from different engines so the
    # ~0.7us descriptor generations overlap instead of serializing.
    nc.gpsimd.dma_start(out=diag_out[:, 8:12], in_=diag[:, 8:12])
    nc.sync.dma_start(out=sums_out[:, 4:8, :], in_=sums[:, 4:8, :])
    ctx.close()


_NC_CACHE = {}


def _get_nc():
    if "nc" not in _NC_CACHE:
        _NC_CACHE["nc"] = _build_bass()
    return _NC_CACHE["nc"]


def _make_in_maps(z1, z2):
    z1 = np.asarray(z1, dtype=np.float32)
    z2 = np.asarray(z2, dtype=np.float32)
    z = np.concatenate([z1, z2], axis=0)  # [8192, 1024]
    nrm = np.sqrt(np.sum(z * z, axis=1, keepdims=True, dtype=np.float32))
    n = z / np.maximum(nrm, EPS)
    repsT = np.ascontiguousarray(n.T * FP8_SCALE).astype(_FP8_NP)  # [1024, 8192]
    in_maps = []
    for c in range(NCORES):
        rolled = np.concatenate([repsT[:, c * BLK :], repsT[:, : c * BLK]], axis=1)
        aT = np.concatenate(
            [repsT[:, c * BLK : (c + 1) * BLK],
             repsT[:, (c + 8) * BLK : (c + 9) * BLK]], axis=1)  # [1024, 1024]
        # kt-pair slabs: [slab, p, j, m]
        a_blk = np.ascontiguousarray(
            aT.reshape(4, 2, P, 2 * BLK).transpose(0, 2, 1, 3))
        # per-chunk: [ch, p, kt, col]
        b_blk = np.ascontiguousarray(
            rolled.reshape(K_TILES, P, N_CHUNKS, BLK).transpose(2, 1, 0, 3))
        in_maps.append({"lhst": a_blk, "brot": b_blk})
    return in_maps


def _combine(results):
    # Per row i: T = rowsum - e_self + e_pos; loss_row = ln(T) - s*pos.
    # O(1M) flops; done in f64.
    rowsum = np.zeros(S, dtype=np.float64)
    selfraw = np.zeros(S, dtype=np.float64)
    posraw = np.zeros(S, dtype=np.float64)
    p = np.arange(P)
    for c, r in enumerate(results):
        sums = r["sums"].astype(np.float64)    # [128, 8, 9]
        diag = r["diag"].astype(np.float64)    # [128, 12]
        cols = r["bsums"].astype(np.float64).sum(axis=1)  # [15, 512]
        for mt in range(4):
            rA = c * BLK + mt * P + p
            rB = (c + 8) * BLK + mt * P + p
            rowsum[rA] += sums[:, mt, 0:9].sum(axis=1)
            rowsum[rB] += sums[:, 4 + mt, 0:8].sum(axis=1)
            selfraw[rA] = diag[:, mt]
            posraw[rA] = diag[:, 4 + mt]
            posraw[rA + B] = diag[:, 4 + mt]
            selfraw[rB] = diag[:, 8 + mt]
        for ch in range(1, 16):
            tb = (c + ch) % GRID
            rowsum[tb * BLK : (tb + 1) * BLK] += cols[ch - 1]
    T = rowsum - np.exp(SIM_SCALE * selfraw) + np.exp(SIM_SCALE * posraw)
    loss_rows = np.log(T) - SIM_SCALE * posraw
    return np.array(loss_rows.mean(), dtype=np.float32)


def run_traced(z1, z2, **spmd_kwargs):
    """Run on HW with profiling; returns (loss, BassKernelResults)."""
    nc = _get_nc()
    in_maps = _make_in_maps(z1, z2)
    res = bass_utils.run_bass_kernel_spmd(
        nc, in_maps, core_ids=list(range(NCORES)), trace=True, **spmd_kwargs
    )
    return _combine(res.results), res


def kernel(z1, z2):
    nc = _get_nc()
    in_maps = _make_in_maps(z1, z2)
    last_err = None
    for _attempt in range(3):
        try:
            res = bass_utils.run_bass_kernel_spmd(
                nc, in_maps, core_ids=list(range(NCORES))
            )
            return _combine(res.results)
        except Exception as e:  # transient device wedge: retry
            last_err = e
            time.sleep(2.0)
    raise last_err


# revision 24
# speedup vs baseline: 1.0468x; 1.0468x over previous
"""Contrastive loss (SimCLR-style NT-Xent) Trainium2 kernel.

Full inputs z1, z2: [4096, 1024] f32. Output: scalar f32 loss.

Strategy (8 NeuronCores, SPMD, no collectives) — SYMMETRIC-TRIANGLE:
  sim = reps @ reps.T is symmetric, so only the upper triangle of the
  16x16 grid of 512x512 blocks is computed (136 blocks total, 17/core):
  each computed off-diagonal block (bi, bj) serves row-block bi via
  per-row exp sums AND row-block bj via per-column exp sums. This
  nearly halves PE work vs the full row-sharded GEMM.

  - Host: L2-normalize rows of reps = concat(z1, z2) [8192, 1024] f32,
    transpose to repsT [1024, 8192], scale by 256, cast fp8e4m3.
  - Core c owns row-blocks {c, c+8} (512 rows each -> groups A, B). Its
    moving operand is repsT with columns rotated by c*512; in rotated
    coords every core computes the same block positions (SPMD):
      group A (rows c):   column chunks 0..8  (chunk 0 = self-diagonal
                          block; chunk 8 = block (c, c+8), whose local
                          diagonal holds the positive pairs)
      group B (rows c+8): column chunks 8..15 (chunk 8 = self-diagonal)
    Pair coverage: d = bj - bi mod 16 in {0..7} for every bi plus d=8
    for bi < 8 covers each unordered block pair exactly once.
  - Per (block, m-tile of 128 rows): 4 fp8 DoubleRow matmuls (K=1024)
    into one PSUM bank; ACT exp(s*x) to bf16 (bias 0: off-diag values
    land in [e^-2, e^2]). Engine split keeps the PE the only bottleneck:
      ACT: 68 exps only (no accumulator reads).
      DVE: per-row sums (reduce over the 512 columns) of every exp tile;
           for off-diagonal blocks also the 3 adds collapsing the 4 exp
           tiles to one [128, 512] block sum, DMA'd out in bf16 — the
           cross-partition (column) reduction happens on the host in f64.
      PE:  GEMM matmuls only, warmed up with dummy f32 matmuls during
           the initial DMA wait so the p-state ramp happens off-line.
  - Diagonals (self A, positive, self B) extracted raw from PSUM on DVE
    (identity mul + reduce).
  - Schedule 0..8(A), 9..15(B), then the chunk-8 B diagonal block last
    (its b chunk loaded twice): the final block emits no column work, so
    only its own 4 exps + row reduces trail the last matmul.
  - Host: assemble per-row totals T = rowsum - e_self + e_pos in f64,
    loss = mean(ln T - s*pos).
"""

import time
from contextlib import ExitStack

import numpy as np
import ml_dtypes

import concourse.bass as bass
import concourse.tile as tile
from concourse import bacc
from concourse import mybir
from concourse import bass_utils

B = 4096
D = 1024
S = 2 * B            # 8192 rows/cols of sim
NCORES = 8
P = 128
BLK = 512            # block edge (= one PSUM bank of f32)
GRID = S // BLK      # 16
K_TILES = D // P     # 8
N_CHUNKS = GRID      # 16 column chunks of 512
INV_T = 10.0         # 1 / temperature
EPS = 1e-12
FP8_SCALE = 256.0    # input scale: keeps fp8e4m3 operands in their sweet spot
SIM_SCALE = INV_T / (FP8_SCALE * FP8_SCALE)  # exp(SIM_SCALE * raw)
N_WARMUP = 5         # dummy matmuls to ramp the PE p-state during DMA wait

_FP32 = mybir.dt.float32
_FP8 = mybir.dt.float8e4
_BF16 = mybir.dt.bfloat16
_FP8_NP = mybir.dt.np(_FP8)
_BF16_NP = mybir.dt.np(_BF16)


def _build_bass():
    # Bacc (not raw Bass): its compile() runs generate_event_semaphores,
    # which splits multi-semaphore waits into standalone EventSemaphore
    # instructions — engine instructions can encode only one wait.
    nc = bacc.Bacc("TRN2", debug=False, num_devices=NCORES, enable_partition_id=False)
    # Stationary rows (A|B), blocked per kt-pair slab on the host:
    # [slab, p, j, m] so each partition reads 2KB contiguous per slab and
    # the PE can start after slab 0 + the first b piece land.
    a_in = nc.dram_tensor(
        "lhst", [K_TILES // 2, P, 2, 2 * BLK], _FP8, kind="ExternalInput"
    ).ap()
    # Rotated moving operand blocked per 512-col chunk: [ch, p, kt, col],
    # 4KB contiguous per partition per chunk.
    b_in = nc.dram_tensor(
        "brot", [N_CHUNKS, P, K_TILES, BLK], _FP8, kind="ExternalInput"
    ).ap()
    # Raw reductions out; the final combine runs on the host.
    sums_out = nc.dram_tensor("sums", [P, 8, 9], _FP32, kind="ExternalOutput").ap()
    diag_out = nc.dram_tensor("diag", [P, 12], _FP32, kind="ExternalOutput").ap()
    # Off-diagonal block sums (exp tiles summed over the 4 m-tiles);
    # the host reduces over the partition axis.
    bsums_out = nc.dram_tensor(
        "bsums", [15, P, BLK], _BF16, kind="ExternalOutput"
    ).ap()

    # Pre-TileContext const region (same pattern as Bass.__init__'s
    # const_aps): values read by hot-loop instructions with no tracked
    # dependency, so they add no per-instruction sync waits. Hand off with
    # one semaphore to the consumers instead of a full barrier.
    bias_th = nc.alloc_sbuf_tensor("const-f32-zero", [P, 1], _FP32)
    nc.gpsimd.memset(bias_th.ap(), 0.0)
    nc.const_aps.aps[(_FP32, 0.0)] = bias_th.ap()
    ident_th = nc.alloc_sbuf_tensor("identity-f32", [P, P], _FP32)
    nc.gpsimd.memset(ident_th.ap(), 0.0)
    ident_inst = nc.gpsimd.affine_select(
        out=ident_th.ap(),
        in_=ident_th.ap(),
        compare_op=mybir.AluOpType.not_equal,
        fill=1.0,
        base=0,
        pattern=[[-1, P]],
        channel_multiplier=1,
    )
    const_sem = nc.alloc_semaphore("const-ready")
    ident_inst.then_inc(const_sem, 1)
    nc.vector.wait_ge(const_sem, 1)
    nc.scalar.wait_ge(const_sem, 1)
    nc.tensor.wait_ge(const_sem, 1)

    with tile.TileContext(nc) as tc:
        _body(tc, a_in, b_in, sums_out, diag_out, bsums_out, ident_th.ap())
    nc.compile()
    return nc


def _body(tc, a_in, b_in, sums_out, diag_out, bsums_out, ident):
    nc = tc.nc
    AF = mybir.ActivationFunctionType

    ctx = ExitStack()
    singles = ctx.enter_context(tc.tile_pool(name="singles", bufs=1))
    bpool = ctx.enter_context(tc.tile_pool(name="bchunks", bufs=3))
    # 4 GEMM banks: deep PSUM pipeline so matmuls never wait on the
    # ACT/DVE consumers of the bank being recycled.
    pspool = ctx.enter_context(tc.tile_pool(name="psum", bufs=4, space="PSUM"))
    wppool = ctx.enter_context(tc.tile_pool(name="warm", bufs=1, space="PSUM"))
    # Exp tiles: consumed by the DVE row-reduce and (off-diag) the adds.
    epool = ctx.enter_context(tc.tile_pool(name="exps", bufs=8))
    # Pairwise add scratch + per-block sums (live until their DMA).
    apool = ctx.enter_context(tc.tile_pool(name="eadds", bufs=4))
    smpool = ctx.enter_context(tc.tile_pool(name="esums", bufs=3))
    scratch = ctx.enter_context(tc.tile_pool(name="scratch", bufs=8))

    # PE p-state warmup: dummy f32 matmuls on the identity const while the
    # first input DMAs are in flight (output never read). Sized to end
    # right as the first inputs land so the ramp never restarts.
    warm_ps = wppool.tile([P, P], _FP32)
    for _ in range(N_WARMUP):
        nc.tensor.matmul(warm_ps, ident, ident, start=True, stop=True)

    # Resident stationary operand: the core's 1024 rows (A|B) transposed,
    # [p, kt, m]; loaded as 2 half-K slabs so matmuls (kt 0-3) start as
    # soon as the first halves land.
    a_t = singles.tile([P, K_TILES, 2 * BLK], _FP8)
    a_view = a_in.rearrange("s p j m -> p s j m")  # [128, 4, 2, 1024]

    # Per-row partial sums: [p, gmt, slot]; group A (gmt 0-3) slots 0..8
    # = chunks 0..8, group B (gmt 4-7) slots 0..7 = chunks 8..15.
    sums = singles.tile([P, 8, 9], _FP32)
    # Raw (pre-exp, scaled) diagonals: cols 0-3 self A, 4-7 positive,
    # 8-11 self B (by m-tile).
    diag = singles.tile([P, 12], _FP32)

    # Schedule: A-group chunks 0..8, B-group 9..15, then the B diagonal
    # block (chunk 8, loaded a second time) LAST — diag blocks emit no
    # column work, so only their own exp/reduce chain trails the PE.
    schedule = [(ch, 0) for ch in range(9)] + \
               [(ch, 1) for ch in range(9, 16)] + [(8, 1)]
    for step, (ch, g) in enumerate(schedule):
        b_t = bpool.tile([P, K_TILES, BLK], _FP8)
        if step == 0:
            # The PE's K-sweep needs (b piece s, a slab s) pairs in order;
            # descriptor generation is ~0.7us serial per dma_start per
            # engine, so spread the issues across three engines and keep
            # the pieces fine-grained (DMA streams ~260GB/s: the first
            # matmul should wait on 320KB, not on a 1.25MB working set).
            nc.sync.dma_start(out=b_t[:, 0:2, :], in_=b_in[0][:, 0:2, :])
            nc.scalar.dma_start(out=b_t[:, 2:4, :], in_=b_in[0][:, 2:4, :])
            nc.gpsimd.dma_start(out=a_t[:, 0:2, :], in_=a_view[:, 0])
            nc.gpsimd.dma_start(out=a_t[:, 2:4, :], in_=a_view[:, 1])
            nc.sync.dma_start(out=b_t[:, 4:6, :], in_=b_in[0][:, 4:6, :])
            nc.sync.dma_start(out=a_t[:, 4:6, :], in_=a_view[:, 2])
            nc.sync.dma_start(out=b_t[:, 6:8, :], in_=b_in[0][:, 6:8, :])
            nc.sync.dma_start(out=a_t[:, 6:8, :], in_=a_view[:, 3])
        else:
            nc.sync.dma_start(out=b_t, in_=b_in[ch])
        if step == 9:
            # Group A fully reduced (steps 0-8): ship its sums and diags
            # while group B runs (issue cost hides under compute).
            nc.sync.dma_start(out=diag_out[:, 0:8], in_=diag[:, 0:8])
            nc.sync.dma_start(out=sums_out[:, 0:4, :], in_=sums[:, 0:4, :])
        goff = g * BLK
        is_diag = (ch == 0 and g == 0) or (ch == 8 and g == 1)
        e_ts = []
        for mt in range(4):
            ps = pspool.tile([P, BLK], _FP32)
            for s in range(4):
                nc.tensor.matmul(
                    ps,
                    a_t[:, 2 * s : 2 * s + 2, goff + mt * P : goff + (mt + 1) * P],
                    b_t[:, 2 * s : 2 * s + 2, :],
                    start=(s == 0),
                    stop=(s == 3),
                    perf_mode=mybir.MatmulPerfMode.DoubleRow,
                )
            gmt = g * 4 + mt
            slot = ch if g == 0 else ch - 8
            e_t = epool.tile([P, BLK], _BF16)
            # Row sums: the ACT accumulator (+187ns/tile) and DVE reduce
            # (~465ns/tile) split the 68 tiles so both engines stay well
            # under the PE; the last blocks use ACT so no DVE backlog
            # trails the final matmul.
            if step in (0, 13, 14, 15, 16):
                nc.scalar.activation(
                    out=e_t, in_=ps, func=AF.Exp, bias=0.0, scale=SIM_SCALE,
                    accum_out=sums[:, gmt, slot : slot + 1])
            else:
                nc.scalar.activation(
                    out=e_t, in_=ps, func=AF.Exp, bias=0.0, scale=SIM_SCALE)
                nc.vector.reduce_sum(
                    sums[:, gmt, slot : slot + 1], e_t,
                    axis=mybir.AxisListType.X)
            e_ts.append(e_t)
            # Raw diagonal extraction on DVE straight from PSUM (fused
            # identity-mask multiply + row reduce): the self diagonals
            # (diag blocks) and the positive diagonal (chunk 8, group A).
            dcol = None
            if is_diag:
                dcol = (0 if ch == 0 else 8) + mt
            elif ch == 8 and g == 0:
                dcol = 4 + mt
            if dcol is not None:
                off = mt * P
                diag_t = scratch.tile([P, P], _FP32)
                nc.vector.tensor_mul(diag_t, ps[:, off : off + P], ident)
                nc.vector.reduce_sum(
                    diag[:, dcol : dcol + 1], diag_t, axis=mybir.AxisListType.X
                )
        if not is_diag:
            # Column contribution of this block (rows of block ch+c): sum
            # the 4 exp tiles on DVE, ship [128, 512] bf16; the host does
            # the final partition reduce in f64.
            e01 = apool.tile([P, BLK], _BF16)
            e23 = apool.tile([P, BLK], _BF16)
            esum = smpool.tile([P, BLK], _BF16)
            nc.vector.tensor_add(e01, e_ts[0], e_ts[1])
            nc.vector.tensor_add(e23, e_ts[2], e_ts[3])
            nc.vector.tensor_add(esum, e01, e23)
            # Issue from GpSimd (idle): on Sync this wait-then-generate
            # would stall the later b-chunk loads behind the DVE adds.
            nc.gpsimd.dma_start(out=bsums_out[ch - 1], in_=esum)

    # Tail: the remaining outputs, issued from different engines so the
    # ~0.7us descriptor generations overlap instead of serializing.
    nc.gpsimd.dma_start(out=diag_out[:, 8:12], in_=diag[:, 8:12])
    nc.sync.dma_start(out=sums_out[:, 4:8, :], in_=sums[:, 4:8, :])
    ctx.close()


_NC_CACHE = {}


def _get_nc():
    if "nc" not in _NC_CACHE:
        _NC_CACHE["nc"] = _build_bass()
    return _NC_CACHE["nc"]


def _make_in_maps(z1, z2):
    z1 = np.asarray(z1, dtype=np.float32)
    z2 = np.asarray(z2, dtype=np.float32)
    z = np.concatenate([z1, z2], axis=0)  # [8192, 1024]
    nrm = np.sqrt(np.sum(z * z, axis=1, keepdims=True, dtype=np.float32))
    n = z / np.maximum(nrm, EPS)
    repsT = np.ascontiguousarray(n.T * FP8_SCALE).astype(_FP8_NP)  # [1024, 8192]
    in_maps = []
    for c in range(NCORES):
        rolled = np.concatenate([repsT[:, c * BLK :], repsT[:, : c * BLK]], axis=1)
        aT = np.concatenate(
            [repsT[:, c * BLK : (c + 1) * BLK],
             repsT[:, (c + 8) * BLK : (c + 9) * BLK]], axis=1)  # [1024, 1024]
        # kt-pair slabs: [slab, p, j, m]
        a_blk = np.ascontiguousarray(
            aT.reshape(4, 2, P, 2 * BLK).transpose(0, 2, 1, 3))
        # per-chunk: [ch, p, kt, col]
        b_blk = np.ascontiguousarray(
            rolled.reshape(K_TILES, P, N_CHUNKS, BLK).transpose(2, 1, 0, 3))
        in_maps.append({"lhst": a_blk, "brot": b_blk})
    return in_maps


def _combine(results):
    # Per row i: T = rowsum - e_self + e_pos; loss_row = ln(T) - s*pos.
    # O(1M) flops; done in f64.
    rowsum = np.zeros(S, dtype=np.float64)
    selfraw = np.zeros(S, dtype=np.float64)
    posraw = np.zeros(S, dtype=np.float64)
    p = np.arange(P)
    for c, r in enumerate(results):
        sums = r["sums"].astype(np.float64)    # [128, 8, 9]
        diag = r["diag"].astype(np.float64)    # [128, 12]
        cols = r["bsums"].astype(np.float64).sum(axis=1)  # [15, 512]
        for mt in range(4):
            rA = c * BLK + mt * P + p
            rB = (c + 8) * BLK + mt * P + p
            rowsum[rA] += sums[:, mt, 0:9].sum(axis=1)
            rowsum[rB] += sums[:, 4 + mt, 0:8].sum(axis=1)
            selfraw[rA] = diag[:, mt]
            posraw[rA] = diag[:, 4 + mt]
            posraw[rA + B] = diag[:, 4 + mt]
            selfraw[rB] = diag[:, 8 + mt]
        for ch in range(1, 16):
            tb = (c + ch) % GRID
            rowsum[tb * BLK : (tb + 1) * BLK] += cols[ch - 1]
    T = rowsum - np.exp(SIM_SCALE * selfraw) + np.exp(SIM_SCALE * posraw)
    loss_rows = np.log(T) - SIM_SCALE * posraw
    return np.array(loss_rows.mean(), dtype=np.float32)


def run_traced(z1, z2, **spmd_kwargs):
    """Run on HW with profiling; returns (loss, BassKernelResults)."""
    nc = _get_nc()
    in_maps = _make_in_maps(z1, z2)
    res = bass_utils.run_bass_kernel_spmd(
        nc, in_maps, core_ids=list(range(NCORES)), trace=True, **spmd_kwargs
    )
    return _combine(res.results), res


def kernel(z1, z2):
    nc = _get_nc()
    in_maps = _make_in_maps(z1, z2)
    last_err = None
    for _attempt in range(3):
        try:
            res = bass_utils.run_bass_kernel_spmd(
                nc, in_maps, core_ids=list(range(NCORES))
            )
            return _combine(res.results)
        except Exception as e:  # transient device wedge: retry
            last_err = e
            time.sleep(2.0)
    raise last_err


# revision 26
# speedup vs baseline: 1.0498x; 1.0029x over previous
"""Contrastive loss (SimCLR-style NT-Xent) Trainium2 kernel.

Full inputs z1, z2: [4096, 1024] f32. Output: scalar f32 loss.

Strategy (8 NeuronCores, SPMD, no collectives) — SYMMETRIC-TRIANGLE:
  sim = reps @ reps.T is symmetric, so only the upper triangle of the
  16x16 grid of 512x512 blocks is computed (136 blocks total, 17/core):
  each computed off-diagonal block (bi, bj) serves row-block bi via
  per-row exp sums AND row-block bj via per-column exp sums. This
  nearly halves PE work vs the full row-sharded GEMM.

  - Host: L2-normalize rows of reps = concat(z1, z2) [8192, 1024] f32,
    transpose to repsT [1024, 8192], scale by 256, cast fp8e4m3.
  - Core c owns row-blocks {c, c+8} (512 rows each -> groups A, B). Its
    moving operand is repsT with columns rotated by c*512; in rotated
    coords every core computes the same block positions (SPMD):
      group A (rows c):   column chunks 0..8  (chunk 0 = self-diagonal
                          block; chunk 8 = block (c, c+8), whose local
                          diagonal holds the positive pairs)
      group B (rows c+8): column chunks 8..15 (chunk 8 = self-diagonal)
    Pair coverage: d = bj - bi mod 16 in {0..7} for every bi plus d=8
    for bi < 8 covers each unordered block pair exactly once.
  - Per (block, m-tile of 128 rows): 4 fp8 DoubleRow matmuls (K=1024)
    into one PSUM bank; ACT exp(s*x) to bf16 (bias 0: off-diag values
    land in [e^-2, e^2]). Engine split keeps the PE the only bottleneck:
      ACT: 68 exps only (no accumulator reads).
      DVE: per-row sums (reduce over the 512 columns) of every exp tile;
           for off-diagonal blocks also the 3 adds collapsing the 4 exp
           tiles to one [128, 512] block sum, DMA'd out in bf16 — the
           cross-partition (column) reduction happens on the host in f64.
      PE:  GEMM matmuls only, warmed up with dummy f32 matmuls during
           the initial DMA wait so the p-state ramp happens off-line.
  - Diagonals (self A, positive, self B) extracted raw from PSUM on DVE
    (identity mul + reduce).
  - Schedule 0..8(A), 9..15(B), then the chunk-8 B diagonal block last
    (its b chunk loaded twice): the final block emits no column work, so
    only its own 4 exps + row reduces trail the last matmul.
  - Host: assemble per-row totals T = rowsum - e_self + e_pos in f64,
    loss = mean(ln T - s*pos).
"""

import time
from contextlib import ExitStack

import numpy as np
import ml_dtypes

import concourse.bass as bass
import concourse.tile as tile
from concourse import bacc
from concourse import mybir
from concourse import bass_utils

B = 4096
D = 1024
S = 2 * B            # 8192 rows/cols of sim
NCORES = 8
P = 128
BLK = 512            # block edge (= one PSUM bank of f32)
GRID = S // BLK      # 16
K_TILES = D // P     # 8
N_CHUNKS = GRID      # 16 column chunks of 512
INV_T = 10.0         # 1 / temperature
EPS = 1e-12
FP8_SCALE = 256.0    # input scale: keeps fp8e4m3 operands in their sweet spot
SIM_SCALE = INV_T / (FP8_SCALE * FP8_SCALE)  # exp(SIM_SCALE * raw)
N_WARMUP = 5         # dummy matmuls to ramp the PE p-state during DMA wait

_FP32 = mybir.dt.float32
_FP8 = mybir.dt.float8e4
_BF16 = mybir.dt.bfloat16
_FP8_NP = mybir.dt.np(_FP8)
_BF16_NP = mybir.dt.np(_BF16)


def _build_bass():
    # Bacc (not raw Bass): its compile() runs generate_event_semaphores,
    # which splits multi-semaphore waits into standalone EventSemaphore
    # instructions — engine instructions can encode only one wait.
    nc = bacc.Bacc("TRN2", debug=False, num_devices=NCORES, enable_partition_id=False)
    # Stationary rows (A|B), blocked per kt-pair slab on the host:
    # [slab, p, j, m] so each partition reads 2KB contiguous per slab and
    # the PE can start after slab 0 + the first b piece land.
    a_in = nc.dram_tensor(
        "lhst", [K_TILES // 2, P, 2, 2 * BLK], _FP8, kind="ExternalInput"
    ).ap()
    # Rotated moving operand blocked per 512-col chunk: [ch, p, kt, col],
    # 4KB contiguous per partition per chunk.
    b_in = nc.dram_tensor(
        "brot", [N_CHUNKS, P, K_TILES, BLK], _FP8, kind="ExternalInput"
    ).ap()
    # Raw reductions out; the final combine runs on the host.
    sums_out = nc.dram_tensor("sums", [P, 8, 9], _FP32, kind="ExternalOutput").ap()
    diag_out = nc.dram_tensor("diag", [P, 12], _FP32, kind="ExternalOutput").ap()
    # Off-diagonal block sums (exp tiles summed over the 4 m-tiles);
    # the host reduces over the partition axis.
    bsums_out = nc.dram_tensor(
        "bsums", [15, P, BLK], _BF16, kind="ExternalOutput"
    ).ap()

    # Pre-TileContext const region (same pattern as Bass.__init__'s
    # const_aps): values read by hot-loop instructions with no tracked
    # dependency, so they add no per-instruction sync waits. Hand off with
    # one semaphore to the consumers instead of a full barrier.
    bias_th = nc.alloc_sbuf_tensor("const-f32-zero", [P, 1], _FP32)
    nc.gpsimd.memset(bias_th.ap(), 0.0)
    nc.const_aps.aps[(_FP32, 0.0)] = bias_th.ap()
    ident_th = nc.alloc_sbuf_tensor("identity-f32", [P, P], _FP32)
    nc.gpsimd.memset(ident_th.ap(), 0.0)
    ident_inst = nc.gpsimd.affine_select(
        out=ident_th.ap(),
        in_=ident_th.ap(),
        compare_op=mybir.AluOpType.not_equal,
        fill=1.0,
        base=0,
        pattern=[[-1, P]],
        channel_multiplier=1,
    )
    const_sem = nc.alloc_semaphore("const-ready")
    ident_inst.then_inc(const_sem, 1)
    nc.vector.wait_ge(const_sem, 1)
    nc.scalar.wait_ge(const_sem, 1)
    nc.tensor.wait_ge(const_sem, 1)

    with tile.TileContext(nc) as tc:
        _body(tc, a_in, b_in, sums_out, diag_out, bsums_out, ident_th.ap())
    nc.compile()
    return nc


def _body(tc, a_in, b_in, sums_out, diag_out, bsums_out, ident):
    nc = tc.nc
    AF = mybir.ActivationFunctionType

    ctx = ExitStack()
    singles = ctx.enter_context(tc.tile_pool(name="singles", bufs=1))
    bpool = ctx.enter_context(tc.tile_pool(name="bchunks", bufs=3))
    # 4 GEMM banks: deep PSUM pipeline so matmuls never wait on the
    # ACT/DVE consumers of the bank being recycled.
    pspool = ctx.enter_context(tc.tile_pool(name="psum", bufs=4, space="PSUM"))
    wppool = ctx.enter_context(tc.tile_pool(name="warm", bufs=1, space="PSUM"))
    # Exp tiles: consumed by the DVE row-reduce and (off-diag) the adds.
    epool = ctx.enter_context(tc.tile_pool(name="exps", bufs=8))
    # Pairwise add scratch + per-block sums (live until their DMA).
    apool = ctx.enter_context(tc.tile_pool(name="eadds", bufs=4))
    smpool = ctx.enter_context(tc.tile_pool(name="esums", bufs=3))
    scratch = ctx.enter_context(tc.tile_pool(name="scratch", bufs=8))

    # PE p-state warmup: dummy f32 matmuls on the identity const while the
    # first input DMAs are in flight (output never read). Sized to end
    # right as the first inputs land so the ramp never restarts.
    warm_ps = wppool.tile([P, P], _FP32)
    for _ in range(N_WARMUP):
        nc.tensor.matmul(warm_ps, ident, ident, start=True, stop=True)

    # Resident stationary operand: the core's 1024 rows (A|B) transposed,
    # [p, kt, m]; loaded as 2 half-K slabs so matmuls (kt 0-3) start as
    # soon as the first halves land.
    a_t = singles.tile([P, K_TILES, 2 * BLK], _FP8)
    a_view = a_in.rearrange("s p j m -> p s j m")  # [128, 4, 2, 1024]

    # Per-row partial sums: [p, gmt, slot]; group A (gmt 0-3) slots 0..8
    # = chunks 0..8, group B (gmt 4-7) slots 0..7 = chunks 8..15.
    sums = singles.tile([P, 8, 9], _FP32)
    # Raw (pre-exp, scaled) diagonals: cols 0-3 self A, 4-7 positive,
    # 8-11 self B (by m-tile).
    diag = singles.tile([P, 12], _FP32)

    # Schedule: A-group chunks 0..8, B-group 9..15, then the B diagonal
    # block (chunk 8, loaded a second time) LAST — diag blocks emit no
    # column work, so only their own exp/reduce chain trails the PE.
    schedule = [(ch, 0) for ch in range(9)] + \
               [(ch, 1) for ch in range(9, 16)] + [(8, 1)]
    for step, (ch, g) in enumerate(schedule):
        b_t = bpool.tile([P, K_TILES, BLK], _FP8)
        if step == 0:
            # The PE's K-sweep needs (b piece s, a slab s) pairs in order;
            # descriptor generation is ~0.7us serial per dma_start per
            # engine, so spread the issues across three engines and keep
            # the pieces fine-grained (DMA streams ~260GB/s: the first
            # matmul should wait on ~100KB, not on a 1.25MB working set).
            # Group B's half of `a` is not needed until step 9 — its four
            # pieces load in the background from GpSimd.
            nc.sync.dma_start(out=b_t[:, 0:2, :], in_=b_in[0][:, 0:2, :])
            nc.gpsimd.dma_start(
                out=a_t[:, 0:2, 0:BLK], in_=a_view[:, 0, :, 0:BLK])
            nc.scalar.dma_start(out=b_t[:, 2:4, :], in_=b_in[0][:, 2:4, :])
            nc.scalar.dma_start(
                out=a_t[:, 2:4, 0:BLK], in_=a_view[:, 1, :, 0:BLK])
            nc.sync.dma_start(
                out=a_t[:, 4:6, 0:BLK], in_=a_view[:, 2, :, 0:BLK])
            nc.sync.dma_start(out=b_t[:, 4:6, :], in_=b_in[0][:, 4:6, :])
            nc.sync.dma_start(
                out=a_t[:, 6:8, 0:BLK], in_=a_view[:, 3, :, 0:BLK])
            nc.sync.dma_start(out=b_t[:, 6:8, :], in_=b_in[0][:, 6:8, :])
            for s in range(4):
                nc.gpsimd.dma_start(
                    out=a_t[:, 2 * s : 2 * s + 2, BLK : 2 * BLK],
                    in_=a_view[:, s, :, BLK : 2 * BLK],
                )
        else:
            nc.sync.dma_start(out=b_t, in_=b_in[ch])
        if step == 9:
            # Group A fully reduced (steps 0-8): ship its sums and diags
            # while group B runs (issue cost hides under compute).
            nc.sync.dma_start(out=diag_out[:, 0:8], in_=diag[:, 0:8])
            nc.sync.dma_start(out=sums_out[:, 0:4, :], in_=sums[:, 0:4, :])
        goff = g * BLK
        is_diag = (ch == 0 and g == 0) or (ch == 8 and g == 1)
        e_ts = []
        for mt in range(4):
            ps = pspool.tile([P, BLK], _FP32)
            for s in range(4):
                nc.tensor.matmul(
                    ps,
                    a_t[:, 2 * s : 2 * s + 2, goff + mt * P : goff + (mt + 1) * P],
                    b_t[:, 2 * s : 2 * s + 2, :],
                    start=(s == 0),
                    stop=(s == 3),
                    perf_mode=mybir.MatmulPerfMode.DoubleRow,
                )
            gmt = g * 4 + mt
            slot = ch if g == 0 else ch - 8
            e_t = epool.tile([P, BLK], _BF16)
            # Row sums: the ACT accumulator (+187ns/tile) and DVE reduce
            # (~465ns/tile) split the 68 tiles so both engines stay well
            # under the PE; the last blocks use ACT so no DVE backlog
            # trails the final matmul.
            if step in (0, 13, 14, 15, 16):
                nc.scalar.activation(
                    out=e_t, in_=ps, func=AF.Exp, bias=0.0, scale=SIM_SCALE,
                    accum_out=sums[:, gmt, slot : slot + 1])
            else:
                nc.scalar.activation(
                    out=e_t, in_=ps, func=AF.Exp, bias=0.0, scale=SIM_SCALE)
                nc.vector.reduce_sum(
                    sums[:, gmt, slot : slot + 1], e_t,
                    axis=mybir.AxisListType.X)
            e_ts.append(e_t)
            # Raw diagonal extraction on DVE straight from PSUM (fused
            # identity-mask multiply + row reduce): the self diagonals
            # (diag blocks) and the positive diagonal (chunk 8, group A).
            dcol = None
            if is_diag:
                dcol = (0 if ch == 0 else 8) + mt
            elif ch == 8 and g == 0:
                dcol = 4 + mt
            if dcol is not None:
                off = mt * P
                diag_t = scratch.tile([P, P], _FP32)
                nc.vector.tensor_mul(diag_t, ps[:, off : off + P], ident)
                nc.vector.reduce_sum(
                    diag[:, dcol : dcol + 1], diag_t, axis=mybir.AxisListType.X
                )
        if not is_diag:
            # Column contribution of this block (rows of block ch+c): sum
            # the 4 exp tiles on DVE, ship [128, 512] bf16; the host does
            # the final partition reduce in f64. The last off-diagonal
            # block (step 15) defers its adds until after the final
            # block's diag extracts so the diag DMA isn't queued behind
            # them on the in-order DVE.
            def emit_cols(e_ts=e_ts, ch=ch, sync_dma=(step == 15)):
                e01 = apool.tile([P, BLK], _BF16)
                e23 = apool.tile([P, BLK], _BF16)
                esum = smpool.tile([P, BLK], _BF16)
                nc.vector.tensor_add(e01, e_ts[0], e_ts[1])
                nc.vector.tensor_add(e23, e_ts[2], e_ts[3])
                nc.vector.tensor_add(esum, e01, e23)
                # GpSimd issues mid-kernel DMAs (on Sync the wait-then-
                # generate would stall later b-chunk loads behind the DVE
                # adds); its slow end-of-program DRAIN (~2.4us) must start
                # early, so the final block's DMA goes on Sync instead.
                eng = nc.sync if sync_dma else nc.gpsimd
                eng.dma_start(out=bsums_out[ch - 1], in_=esum)

            if step == 15:
                deferred_cols = emit_cols
            else:
                emit_cols()

    # Tail on Sync only (GpSimd and ACT must drain early): diag first
    # (ready once the final block's DVE extracts finish), then the
    # deferred column sums, then the B row sums (ready at the last exp).
    nc.sync.dma_start(out=diag_out[:, 8:12], in_=diag[:, 8:12])
    deferred_cols()
    nc.sync.dma_start(out=sums_out[:, 4:8, :], in_=sums[:, 4:8, :])
    ctx.close()


_NC_CACHE = {}


def _get_nc():
    if "nc" not in _NC_CACHE:
        _NC_CACHE["nc"] = _build_bass()
    return _NC_CACHE["nc"]


def _make_in_maps(z1, z2):
    z1 = np.asarray(z1, dtype=np.float32)
    z2 = np.asarray(z2, dtype=np.float32)
    z = np.concatenate([z1, z2], axis=0)  # [8192, 1024]
    nrm = np.sqrt(np.sum(z * z, axis=1, keepdims=True, dtype=np.float32))
    n = z / np.maximum(nrm, EPS)
    repsT = np.ascontiguousarray(n.T * FP8_SCALE).astype(_FP8_NP)  # [1024, 8192]
    in_maps = []
    for c in range(NCORES):
        rolled = np.concatenate([repsT[:, c * BLK :], repsT[:, : c * BLK]], axis=1)
        aT = np.concatenate(
            [repsT[:, c * BLK : (c + 1) * BLK],
             repsT[:, (c + 8) * BLK : (c + 9) * BLK]], axis=1)  # [1024, 1024]
        # kt-pair slabs: [slab, p, j, m]
        a_blk = np.ascontiguousarray(
            aT.reshape(4, 2, P, 2 * BLK).transpose(0, 2, 1, 3))
        # per-chunk: [ch, p, kt, col]
        b_blk = np.ascontiguousarray(
            rolled.reshape(K_TILES, P, N_CHUNKS, BLK).transpose(2, 1, 0, 3))
        in_maps.append({"lhst": a_blk, "brot": b_blk})
    return in_maps


def _combine(results):
    # Per row i: T = rowsum - e_self + e_pos; loss_row = ln(T) - s*pos.
    # O(1M) flops; done in f64.
    rowsum = np.zeros(S, dtype=np.float64)
    selfraw = np.zeros(S, dtype=np.float64)
    posraw = np.zeros(S, dtype=np.float64)
    p = np.arange(P)
    for c, r in enumerate(results):
        sums = r["sums"].astype(np.float64)    # [128, 8, 9]
        diag = r["diag"].astype(np.float64)    # [128, 12]
        cols = r["bsums"].astype(np.float64).sum(axis=1)  # [15, 512]
        for mt in range(4):
            rA = c * BLK + mt * P + p
            rB = (c + 8) * BLK + mt * P + p
            rowsum[rA] += sums[:, mt, 0:9].sum(axis=1)
            rowsum[rB] += sums[:, 4 + mt, 0:8].sum(axis=1)
            selfraw[rA] = diag[:, mt]
            posraw[rA] = diag[:, 4 + mt]
            posraw[rA + B] = diag[:, 4 + mt]
            selfraw[rB] = diag[:, 8 + mt]
        for ch in range(1, 16):
            tb = (c + ch) % GRID
            rowsum[tb * BLK : (tb + 1) * BLK] += cols[ch - 1]
    T = rowsum - np.exp(SIM_SCALE * selfraw) + np.exp(SIM_SCALE * posraw)
    loss_rows = np.log(T) - SIM_SCALE * posraw
    return np.array(loss_rows.mean(), dtype=np.float32)


def run_traced(z1, z2, **spmd_kwargs):
    """Run on HW with profiling; returns (loss, BassKernelResults)."""
    nc = _get_nc()
    in_maps = _make_in_maps(z1, z2)
    res = bass_utils.run_bass_kernel_spmd(
        nc, in_maps, core_ids=list(range(NCORES)), trace=True, **spmd_kwargs
    )
    return _combine(res.results), res


def kernel(z1, z2):
    nc = _get_nc()
    in_maps = _make_in_maps(z1, z2)
    last_err = None
    for _attempt in range(3):
        try:
            res = bass_utils.run_bass_kernel_spmd(
                nc, in_maps, core_ids=list(range(NCORES))
            )
            return _combine(res.results)
        except Exception as e:  # transient device wedge: retry
            last_err = e
            time.sleep(2.0)
    raise last_err
